# revision 16
# baseline (speedup 1.0000x reference)
"""Cosine-similarity retrieval kernel for 8 Trainium2 NeuronCores.

Computes out[n, m] = <x1[n]/||x1[n]||, x2[m]/||x2[m]||> / TEMP for
x1, x2 of shape (8192, 1024) fp32 (output (8192, 8192) fp32).

Sharding: x1 rows data-parallel across the 8 cores (1024-row slabs),
x2 replicated. Each core computes its (1024, 8192) slab of the score
matrix.

Device pipeline (per core), all arithmetic on-device:
  - inputs are uploaded d-major (host transpose only, no host math):
    x1t [d, n_slab], x2t [d, m] fp32; SWDGE DMA casts f32->bf16 on the
    way into SBUF
  - x1 row norms via N=1 matmuls (squared k-tile stationary, ones
    moving) accumulated in one PSUM bank -> per-partition layout; this
    PE work rides the DMA-bound head for free
  - x2 row norms WITHOUT per-k PE matmuls: ACT squares each k-tile,
    DVE tree-adds the 8 bf16 square tiles, then only 2 ones-matmuls
    per 1024-column block reduce across partitions (saves ~24us of PE
    time vs per-k ones-matmuls); tree-add emission is spread across
    the block window so drain STTs are never delayed in the DVE FIFO
  - head DMA order x1a, x2k0, x1b, x2k1..k7 keeps the PE fed with
    x1-norm matmuls while x2 streams; a 1-elem gpsimd copy gates the
    next block's prefetch behind the head's last byte so it cannot
    steal head HBM bandwidth
  - main GEMM: bf16 matmuls, k-accumulated in PSUM, 512-col chunks;
    3 PSUM tiles of runway (head-only banks are reclaimed); each
    512-col half drains (DVE scalar_tensor_tensor applying both norm
    scales) while the PE fills the other bank
"""

import sys

if "/opt/trn_rl_repo" not in sys.path:
    sys.path.insert(0, "/opt/trn_rl_repo")

import numpy as np

TEMP = 0.05
N_CORES = 8

_CACHE = {}


def _build(n_slab, m, d):
    """Build + compile the per-core Bass kernel. Shapes are per-core."""
    from contextlib import ExitStack

    import concourse.mybir as mybir
    import concourse.tile as tile
    from concourse import bacc

    f32 = mybir.dt.float32
    bf16 = mybir.dt.bfloat16
    AF = mybir.ActivationFunctionType
    ALU = mybir.AluOpType

    assert d % 128 == 0 and n_slab % 128 == 0 and m % 1024 == 0
    KT = d // 128          # contraction k-tiles
    NMT = n_slab // 128    # output row tiles
    CB = 1024              # x2 column block per stage-B/C step
    NCB = m // CB
    CHW = 512              # psum chunk width (one PSUM bank)
    KH = max(1, KT // 2)   # k-tile half split for batched DMAs

    nc = bacc.Bacc("TRN2", target_bir_lowering=False, debug=False,
                   num_devices=N_CORES)
    x1t = nc.declare_dram_parameter("x1t", [d, n_slab], f32, isOutput=False)
    x2t = nc.declare_dram_parameter("x2t", [d, m], f32, isOutput=False)
    out = nc.declare_dram_parameter("out", [n_slab, m], f32, isOutput=True)

    # d-major DRAM views with partition dim first: [128, KT, cols]
    x1t_r = x1t.ap().rearrange("(kk p) n -> p kk n", p=128)
    x2t_r = x2t.ap().rearrange("(kk p) mm -> p kk mm", p=128)
    out_ap = out.ap()

    with tile.TileContext(nc) as tc, ExitStack() as ctx:
        resid = ctx.enter_context(tc.tile_pool(name="resid", bufs=1))
        x1n = resid.tile([128, KT, n_slab], bf16)   # bf16 cast of x1t
        srep2 = resid.tile([128, m], f32)           # 1/||x2|| replicated
        n1i = resid.tile([128, NMT], f32)           # (1/TEMP)/||x1|| per-part
        ones = resid.tile([128, 128], bf16)
        nc.vector.memset(ones, 1.0)

        # persistent PSUM: npA/npB (x2 norm ones-matmul outputs) = 2 banks
        normp = ctx.enter_context(tc.tile_pool(name="normp", bufs=1,
                                               space="PSUM"))
        npA = normp.tile([128, CHW], f32, tag="npA", name="npA")
        npB = normp.tile([128, CHW], f32, tag="npB", name="npB")

        x2p = ctx.enter_context(tc.tile_pool(name="x2p", bufs=3))
        sqp = ctx.enter_context(tc.tile_pool(name="sqp", bufs=2))
        trp = ctx.enter_context(tc.tile_pool(name="trp", bufs=2))
        vec = ctx.enter_context(tc.tile_pool(name="vec", bufs=2))

        # preload both ACT table sets (Square, Sqrt) off the critical path
        dum = vec.tile([128, 1], f32, tag="dum", name="dum", bufs=1)
        nc.vector.memset(dum, 1.0)
        dumo = vec.tile([128, 1], f32, tag="dumo", name="dumo", bufs=1)
        nc.scalar.activation(dumo[:], dum[:], AF.Square)
        nc.scalar.activation(dumo[:], dum[:], AF.Sqrt)

        wsrc = vec.tile([128, CHW], bf16, tag="wsrc", name="wsrc", bufs=1)
        nc.vector.memset(wsrc, 0.0)

        def tree_reduce(parts, nm, cnt0):
            """Pairwise DVE adds of [128, CB] bf16 APs -> single AP."""
            lvl = list(parts)
            cnt = cnt0
            while len(lvl) > 1:
                nxt = []
                for j in range(0, len(lvl) - 1, 2):
                    tt = trp.tile([128, CB], bf16, tag=f"tr{cnt}",
                                  name=f"tr{cnt}{nm}")
                    nc.vector.tensor_add(tt[:], lvl[j], lvl[j + 1])
                    nxt.append(tt[:])
                    cnt += 1
                if len(lvl) % 2:
                    nxt.append(lvl[-1])
                lvl = nxt
            return lvl[0]

        def b_norm_reduce(acc, cb, nm):
            """2 ones-matmuls + sqrt + recip -> srep2[:, cb block]."""
            for i, npX in enumerate((npA, npB)):
                nc.tensor.matmul(npX[:], ones[:, :128],
                                 acc[:, i * CHW:(i + 1) * CHW],
                                 start=True, stop=True, skip_group_check=True)
                tmp = vec.tile([128, CHW], f32, tag="vtmp", name=f"bt{nm}_{i}")
                nc.scalar.activation(tmp[:], npX[:], AF.Sqrt)
                nc.vector.reciprocal_approx_fast(
                    out=srep2[:, cb * CB + i * CHW:cb * CB + (i + 1) * CHW],
                    in_=tmp[:])

        # -------------------- head (cb0) --------------------
        x2cb0 = x2p.tile([128, KT, CB], bf16, tag="x2cb", name="x2cb0")
        sq2h = sqp.tile([128, KT, CB], bf16, tag="sq2", name="sq2h")

        with tc.tile_pool(name="headp", bufs=1, space="PSUM") as headp, \
             tc.tile_pool(name="a_sq", bufs=2) as a_sq:
            wps = headp.tile([128, CHW], f32, tag="wps", name="wps")
            np_n1 = headp.tile([128, NMT], f32, tag="np_n1", name="np_n1")

            # HAM warm-up: ~7us of dummy matmuls while the first DMAs
            # stream in, so real matmuls run at the unthrottled clock
            for _ in range(16):
                nc.tensor.matmul(wps[:], ones[:, :128], wsrc[:],
                                 start=True, stop=True,
                                 skip_group_check=True)

            # head DMAs: x1 in 1MB k-pair transfers (fewer issues eases
            # the 8-lane DMAHW rotation; >=1MB keeps SWDGE efficient),
            # x2 per-k 512KB for stage-C arrival granularity; x1 early
            # so its norm matmuls feed the PE while x2 streams
            for k in range(0, KT, 2):
                k1 = min(k + 2, KT)
                nc.gpsimd.dma_start(out=x1n[:, k:k1, :],
                                    in_=x1t_r[:, k:k1, :])
            for k in range(KT):
                nc.gpsimd.dma_start(out=x2cb0[:, k, :], in_=x2t_r[:, k, 0:CB])

            # x1 row norms (N=1 matmuls) + cb0 squares, emission
            # interleaved to match DMA arrival order
            def x1_norm_half(k0, k1):
                for k in range(k0, k1):
                    sq = a_sq.tile([128, n_slab], bf16, tag="a_sq",
                                   name=f"a_sqt{k}")
                    nc.scalar.activation(sq[:], x1n[:, k, :], AF.Square)
                    for mt in range(NMT):
                        nc.tensor.matmul(np_n1[:, mt:mt + 1],
                                         sq[:, mt * 128:(mt + 1) * 128],
                                         ones[:, 0:1],
                                         start=(k == 0 and mt == 0),
                                         stop=(k == KT - 1 and
                                               mt == NMT - 1),
                                         skip_group_check=True)

            def x2sq(k):
                nc.scalar.activation(sq2h[:, k:k + 1, :],
                                     x2cb0[:, k:k + 1, :], AF.Square)

            x1_norm_half(0, KH)
            x2sq(0)
            x1_norm_half(KH, KT)
            tmp8 = vec.tile([128, NMT], f32, tag="tmp8", name="tmp8", bufs=1)
            # sqrt(nsq * TEMP^2) = ||x1||*TEMP ; recip -> (1/TEMP)/||x1||
            nc.scalar.activation(tmp8[:], np_n1[:], AF.Sqrt,
                                 scale=float(TEMP * TEMP))
            nc.vector.reciprocal_approx_fast(out=n1i[:], in_=tmp8[:])

            # cb0 squares k1..k7 with pair-adds as both halves land
            pairs = []
            pcnt = 0
            for k in range(1, KT):
                x2sq(k)
                if k % 2 == 1:
                    tt = trp.tile([128, CB], bf16, tag=f"tr{pcnt}",
                                  name=f"tr{pcnt}h")
                    nc.vector.tensor_add(tt[:], sq2h[:, k - 1, :],
                                         sq2h[:, k, :])
                    pairs.append(tt[:])
                    pcnt += 1
            if KT % 2 == 1:
                pairs.append(sq2h[:, KT - 1, :])
            acc_cb0 = tree_reduce(pairs, "h", pcnt)
        # headp closed: wps + np_n1 banks reclaimed for the GEMM runway

        # ------------- stages B+C over column blocks -------------
        # Iteration cb: DMA(cb+1) at top (gpsimd queue-blocks on the
        # x2p slot -> natural depth-1 prefetch); squares+pair-adds for
        # cb+1 after mt4; final adds after mt5; ones-matmuls+sqrt+recip
        # after mt6. cb0: its own norm reduce runs after mt1's chains,
        # with mt0/mt1 drains deferred until just after it.
        mt_sq = max(0, NMT - 4)
        mt_fin = max(0, NMT - 3)
        mt_red = max(0, NMT - 2)
        mt_flush = min(1, NMT - 1)
        with tc.tile_pool(name="cps", bufs=3, space="PSUM") as cps, \
             tc.tile_pool(name="ost", bufs=10) as ost:

            def _drain(ps, i, mt, csl_base):
                # out = (psum * (1/TEMP)/||x1||_row) * (1/||x2||)_col,
                # 512-col half: drains bank i while PE fills the other
                ot = ost.tile([128, CHW], f32, tag="c_ot", name="c_ot")
                nc.vector.scalar_tensor_tensor(
                    out=ot[:], in0=ps[:, i * CHW:(i + 1) * CHW],
                    scalar=n1i[:, mt:mt + 1],
                    in1=srep2[:, csl_base + i * CHW:csl_base + (i + 1) * CHW],
                    op0=ALU.mult, op1=ALU.mult)
                nc.sync.dma_start(
                    out=out_ap[mt * 128:(mt + 1) * 128,
                               csl_base + i * CHW:csl_base + (i + 1) * CHW],
                    in_=ot[:])

            x2cb = x2cb0
            x2nx = None
            sq_next = None
            pairs_next = None
            acc_next = None
            deferred = []
            for cb in range(NCB):
                csl_base = cb * CB
                if cb < NCB - 1:
                    # chain: a 1-elem gpsimd copy of THIS block's last
                    # k-tile; the next block's dma_starts sit behind it
                    # on the gpsimd queue, so input blocks never compete
                    # with each other for HBM bandwidth
                    g = vec.tile([128, 1], bf16, tag="gate",
                                 name=f"gate{cb}")
                    nc.gpsimd.tensor_copy(g[:], x2cb[:, KT - 1, CB - 1:CB])
                    nsl = slice((cb + 1) * CB, (cb + 2) * CB)
                    x2nx = x2p.tile([128, KT, CB], bf16, tag="x2cb",
                                    name=f"x2cb{cb + 1}")
                    for k in range(KT):
                        nc.gpsimd.dma_start(out=x2nx[:, k, :],
                                            in_=x2t_r[:, k, nsl])
                for mt in range(NMT):
                    ps = cps.tile([128, CB], f32, tag="c_ps", name="c_ps")
                    for i in range(2):
                        for k in range(KT):
                            nc.tensor.matmul(
                                ps[:, i * CHW:(i + 1) * CHW],
                                x1n[:, k, mt * 128:(mt + 1) * 128],
                                x2cb[:, k, i * CHW:(i + 1) * CHW],
                                start=(k == 0), stop=(k == KT - 1))
                        if cb == 0 and mt <= mt_flush:
                            deferred.append((ps, i, mt))
                        else:
                            _drain(ps, i, mt, csl_base)
                    if cb == 0 and mt == mt_flush:
                        # cb0's own column norms, then the deferred drains
                        b_norm_reduce(acc_cb0, 0, "h")
                        for (dps, di, dmt) in deferred:
                            _drain(dps, di, dmt, csl_base)
                        deferred = []
                    if cb < NCB - 1:
                        if mt == mt_sq:
                            # squares + pair-adds for cb+1
                            sq_next = sqp.tile([128, KT, CB], bf16,
                                               tag="sq2",
                                               name=f"sq2b{cb + 1}")
                            nc.scalar.activation(sq_next[:, 0:KH, :],
                                                 x2nx[:, 0:KH, :], AF.Square)
                            if KH < KT:
                                nc.scalar.activation(sq_next[:, KH:KT, :],
                                                     x2nx[:, KH:KT, :],
                                                     AF.Square)
                            pairs_next = []
                            pc = 0
                            for k2 in range(0, KT - 1, 2):
                                tt = trp.tile([128, CB], bf16,
                                              tag=f"tr{pc}",
                                              name=f"tr{pc}b{cb + 1}")
                                nc.vector.tensor_add(tt[:],
                                                     sq_next[:, k2, :],
                                                     sq_next[:, k2 + 1, :])
                                pairs_next.append(tt[:])
                                pc += 1
                            if KT % 2 == 1:
                                pairs_next.append(sq_next[:, KT - 1, :])
                        if mt == mt_fin:
                            # final tree levels for cb+1
                            acc_next = tree_reduce(pairs_next,
                                                   f"b{cb + 1}",
                                                   (KT + 1) // 2)
                        if mt == mt_red:
                            # cb+1's ones-matmuls + sqrt/recip -> srep2
                            b_norm_reduce(acc_next, cb + 1, f"b{cb + 1}")
                if cb < NCB - 1:
                    x2cb = x2nx

    nc.compile()
    return nc


def _get_nc(n_slab, m, d):
    key = (n_slab, m, d)
    if key not in _CACHE:
        _CACHE[key] = _build(n_slab, m, d)
    return _CACHE[key]


def _in_maps(x1, x2, n_slab):
    x1t = np.ascontiguousarray(x1.T)  # [d, n]
    x2t = np.ascontiguousarray(x2.T)  # [d, m]
    return [
        {"x1t": np.ascontiguousarray(x1t[:, i * n_slab:(i + 1) * n_slab]),
         "x2t": x2t}
        for i in range(N_CORES)
    ]


def kernel(x1, x2):
    from concourse.bass_utils import run_bass_kernel_spmd

    x1 = np.asarray(x1, dtype=np.float32)
    x2 = np.asarray(x2, dtype=np.float32)
    n, d = x1.shape
    m, d2 = x2.shape
    assert d == d2 and n % N_CORES == 0
    n_slab = n // N_CORES

    nc = _get_nc(n_slab, m, d)
    res = run_bass_kernel_spmd(nc, _in_maps(x1, x2, n_slab),
                               core_ids=list(range(N_CORES)))
    return np.concatenate([res.results[i]["out"] for i in range(N_CORES)],
                          axis=0)


if __name__ == "__main__":
    # small-shape self test
    rng = np.random.default_rng(0)
    n, m, d = 1024, 2048, 256
    x1 = rng.standard_normal((n, d), dtype=np.float32)
    x2 = rng.standard_normal((m, d), dtype=np.float32)
    got = kernel(x1, x2)
    x1n = x1 / np.linalg.norm(x1, axis=1, keepdims=True)
    x2n = x2 / np.linalg.norm(x2, axis=1, keepdims=True)
    want = (x1n @ x2n.T) / TEMP
    rel = np.linalg.norm(got - want) / np.linalg.norm(want)
    print("rel l2 err:", rel)
    print("max abs err:", np.abs(got - want).max(), "scale:", np.abs(want).max())


# revision 23
# speedup vs baseline: 1.0420x; 1.0420x over previous
"""Cosine-similarity retrieval kernel for 8 Trainium2 NeuronCores.

Computes out[n, m] = <x1[n]/||x1[n]||, x2[m]/||x2[m]||> / TEMP for
x1, x2 of shape (8192, 1024) fp32 (output (8192, 8192) fp32).

Sharding: x1 rows data-parallel across the 8 cores (1024-row slabs),
x2 replicated. Each core computes its (1024, 8192) slab of the score
matrix.

Device pipeline (per core), all arithmetic on-device:
  - inputs are uploaded d-major (host transpose only, no host math):
    x1t [d, n_slab], x2t [d, m] fp32; SWDGE DMA casts f32->bf16 on the
    way into SBUF
  - x1 row norms via N=1 matmuls (squared k-tile stationary, ones
    moving) accumulated in one PSUM bank -> per-partition layout; this
    PE work rides the DMA-bound head for free
  - x2 row norms WITHOUT per-k PE matmuls: ACT squares each k-tile,
    DVE tree-adds the 8 bf16 square tiles, then only 2 ones-matmuls
    per 1024-column block reduce across partitions (saves ~24us of PE
    time vs per-k ones-matmuls); tree-add emission is spread across
    the block window so drain STTs are never delayed in the DVE FIFO
  - head DMA order x1a, x2k0, x1b, x2k1..k7 keeps the PE fed with
    x1-norm matmuls while x2 streams; a 1-elem gpsimd copy gates the
    next block's prefetch behind the head's last byte so it cannot
    steal head HBM bandwidth
  - main GEMM: bf16 matmuls, k-accumulated in PSUM, 512-col chunks;
    3 PSUM tiles of runway (head-only banks are reclaimed); each
    512-col half drains (DVE scalar_tensor_tensor applying both norm
    scales) while the PE fills the other bank
"""

import sys

if "/opt/trn_rl_repo" not in sys.path:
    sys.path.insert(0, "/opt/trn_rl_repo")

import numpy as np

TEMP = 0.05
N_CORES = 8

_CACHE = {}


def _build(n_slab, m, d):
    """Build + compile the per-core Bass kernel. Shapes are per-core."""
    from contextlib import ExitStack

    import concourse.mybir as mybir
    import concourse.tile as tile
    from concourse import bacc

    f32 = mybir.dt.float32
    bf16 = mybir.dt.bfloat16
    AF = mybir.ActivationFunctionType
    ALU = mybir.AluOpType

    assert d % 128 == 0 and n_slab % 128 == 0 and m % 1024 == 0
    KT = d // 128          # contraction k-tiles
    NMT = n_slab // 128    # output row tiles
    CB = 1024              # x2 column block per stage-B/C step
    NCB = m // CB
    CHW = 512              # psum chunk width (one PSUM bank)
    KH = max(1, KT // 2)   # k-tile half split for batched DMAs

    nc = bacc.Bacc("TRN2", target_bir_lowering=False, debug=False,
                   num_devices=N_CORES)
    # x1t: d-major, k-tiles are contiguous 512KB chunks already.
    # x2b: blocked [NCB, KT, 128, CB] so each (cb, k) load is one fully
    # contiguous 512KB read. outb: blocked [NCB, n_slab, CB] so each
    # (cb, mt) store is one fully contiguous 512KB write — the natural
    # [n_slab, m] layout writes 2KB rows at 32KB stride, which thrashes
    # HBM row buffers and caps the write stream at ~150 GB/s. Host
    # reshapes are layout-only.
    x1t = nc.declare_dram_parameter("x1t", [d, n_slab], f32, isOutput=False)
    x2b = nc.declare_dram_parameter("x2b", [NCB, KT, 128, CB], f32,
                                    isOutput=False)
    outb = nc.declare_dram_parameter("outb", [NCB, n_slab, CB], f32,
                                     isOutput=True)

    x1t_r = x1t.ap().rearrange("(kk p) n -> p kk n", p=128)
    x2b_ap = x2b.ap()
    outb_ap = outb.ap()

    with tile.TileContext(nc) as tc, ExitStack() as ctx:
        resid = ctx.enter_context(tc.tile_pool(name="resid", bufs=1))
        x1n = resid.tile([128, KT, n_slab], bf16)   # bf16 cast of x1t
        srep2 = resid.tile([128, m], f32)           # 1/||x2|| replicated
        n1i = resid.tile([128, NMT], f32)           # (1/TEMP)/||x1|| per-part
        ones = resid.tile([128, 128], bf16)
        nc.vector.memset(ones, 1.0)

        # persistent PSUM: npA/npB (x2 norm ones-matmul outputs) = 2 banks
        normp = ctx.enter_context(tc.tile_pool(name="normp", bufs=1,
                                               space="PSUM"))
        npA = normp.tile([128, CHW], f32, tag="npA", name="npA")
        npB = normp.tile([128, CHW], f32, tag="npB", name="npB")

        x2p = ctx.enter_context(tc.tile_pool(name="x2p", bufs=3))
        sqp = ctx.enter_context(tc.tile_pool(name="sqp", bufs=2))
        trp = ctx.enter_context(tc.tile_pool(name="trp", bufs=2))
        vec = ctx.enter_context(tc.tile_pool(name="vec", bufs=2))

        # preload both ACT table sets (Square, Sqrt) off the critical path
        dum = vec.tile([128, 1], f32, tag="dum", name="dum", bufs=1)
        nc.vector.memset(dum, 1.0)
        dumo = vec.tile([128, 1], f32, tag="dumo", name="dumo", bufs=1)
        nc.scalar.activation(dumo[:], dum[:], AF.Square)
        nc.scalar.activation(dumo[:], dum[:], AF.Sqrt)

        wsrc = vec.tile([128, CHW], bf16, tag="wsrc", name="wsrc", bufs=1)
        nc.vector.memset(wsrc, 0.0)

        def tree_reduce(parts, nm, cnt0):
            """Pairwise DVE adds of [128, CB] bf16 APs -> single AP."""
            lvl = list(parts)
            cnt = cnt0
            while len(lvl) > 1:
                nxt = []
                for j in range(0, len(lvl) - 1, 2):
                    tt = trp.tile([128, CB], bf16, tag=f"tr{cnt}",
                                  name=f"tr{cnt}{nm}")
                    nc.vector.tensor_add(tt[:], lvl[j], lvl[j + 1])
                    nxt.append(tt[:])
                    cnt += 1
                if len(lvl) % 2:
                    nxt.append(lvl[-1])
                lvl = nxt
            return lvl[0]

        def b_norm_reduce(acc, cb, nm):
            """2 ones-matmuls + sqrt + recip -> srep2[:, cb block]."""
            for i, npX in enumerate((npA, npB)):
                nc.tensor.matmul(npX[:], ones[:, :128],
                                 acc[:, i * CHW:(i + 1) * CHW],
                                 start=True, stop=True, skip_group_check=True)
                tmp = vec.tile([128, CHW], f32, tag="vtmp", name=f"bt{nm}_{i}")
                nc.scalar.activation(tmp[:], npX[:], AF.Sqrt)
                nc.vector.reciprocal_approx_fast(
                    out=srep2[:, cb * CB + i * CHW:cb * CB + (i + 1) * CHW],
                    in_=tmp[:])

        # -------------------- head (cb0) --------------------
        x2cb0 = x2p.tile([128, KT, CB], bf16, tag="x2cb", name="x2cb0")
        sq2h = sqp.tile([128, KT, CB], bf16, tag="sq2", name="sq2h")

        with tc.tile_pool(name="headp", bufs=1, space="PSUM") as headp, \
             tc.tile_pool(name="a_sq", bufs=2) as a_sq:
            wps = headp.tile([128, CHW], f32, tag="wps", name="wps")
            np_n1 = headp.tile([128, NMT], f32, tag="np_n1", name="np_n1")

            # HAM warm-up: ~7us of dummy matmuls while the first DMAs
            # stream in, so real matmuls run at the unthrottled clock
            for _ in range(16):
                nc.tensor.matmul(wps[:], ones[:, :128], wsrc[:],
                                 start=True, stop=True,
                                 skip_group_check=True)

            # head DMAs: x1 in 1MB k-pair transfers (fewer issues eases
            # the 8-lane DMAHW rotation; >=1MB keeps SWDGE efficient),
            # x2 per-k 512KB for stage-C arrival granularity; x1 early
            # so its norm matmuls feed the PE while x2 streams
            for k in range(0, KT, 2):
                k1 = min(k + 2, KT)
                nc.gpsimd.dma_start(out=x1n[:, k:k1, :],
                                    in_=x1t_r[:, k:k1, :])
            for k in range(KT):
                nc.gpsimd.dma_start(out=x2cb0[:, k, :], in_=x2b_ap[0, k])

            # x1 row norms (N=1 matmuls) + cb0 squares, emission
            # interleaved to match DMA arrival order
            def x1_norm_half(k0, k1):
                for k in range(k0, k1):
                    sq = a_sq.tile([128, n_slab], bf16, tag="a_sq",
                                   name=f"a_sqt{k}")
                    nc.scalar.activation(sq[:], x1n[:, k, :], AF.Square)
                    for mt in range(NMT):
                        nc.tensor.matmul(np_n1[:, mt:mt + 1],
                                         sq[:, mt * 128:(mt + 1) * 128],
                                         ones[:, 0:1],
                                         start=(k == 0 and mt == 0),
                                         stop=(k == KT - 1 and
                                               mt == NMT - 1),
                                         skip_group_check=True)

            def x2sq(k):
                nc.scalar.activation(sq2h[:, k:k + 1, :],
                                     x2cb0[:, k:k + 1, :], AF.Square)

            x1_norm_half(0, KH)
            x2sq(0)
            x1_norm_half(KH, KT)
            tmp8 = vec.tile([128, NMT], f32, tag="tmp8", name="tmp8", bufs=1)
            # sqrt(nsq * TEMP^2) = ||x1||*TEMP ; recip -> (1/TEMP)/||x1||
            nc.scalar.activation(tmp8[:], np_n1[:], AF.Sqrt,
                                 scale=float(TEMP * TEMP))
            nc.vector.reciprocal_approx_fast(out=n1i[:], in_=tmp8[:])

            # cb0 squares k1..k7 with pair-adds as both halves land
            pairs = []
            pcnt = 0
            for k in range(1, KT):
                x2sq(k)
                if k % 2 == 1:
                    tt = trp.tile([128, CB], bf16, tag=f"tr{pcnt}",
                                  name=f"tr{pcnt}h")
                    nc.vector.tensor_add(tt[:], sq2h[:, k - 1, :],
                                         sq2h[:, k, :])
                    pairs.append(tt[:])
                    pcnt += 1
            if KT % 2 == 1:
                pairs.append(sq2h[:, KT - 1, :])
            acc_cb0 = tree_reduce(pairs, "h", pcnt)
        # headp closed: wps + np_n1 banks reclaimed for the GEMM runway

        # ------------- stages B+C over column blocks -------------
        # Iteration cb: DMA(cb+1) at top (gpsimd queue-blocks on the
        # x2p slot -> natural depth-1 prefetch); squares+pair-adds for
        # cb+1 after mt4; final adds after mt5; ones-matmuls+sqrt+recip
        # after mt6. cb0: its own norm reduce runs after mt1's chains,
        # with mt0/mt1 drains deferred until just after it.
        mt_sq = max(0, NMT - 4)
        mt_fin = max(0, NMT - 3)
        mt_red = max(0, NMT - 2)
        mt_flush = min(1, NMT - 1)
        with tc.tile_pool(name="cps", bufs=3, space="PSUM") as cps, \
             tc.tile_pool(name="ost", bufs=5) as ost:

            def _drain(ps, ot, i, mt, cb):
                # out = (psum * (1/TEMP)/||x1||_row) * (1/||x2||)_col,
                # 512-col half: drains bank i while PE fills the other;
                # the store fires once both halves are in ot (one fully
                # contiguous 512KB write to the blocked layout)
                nc.vector.scalar_tensor_tensor(
                    out=ot[:, i * CHW:(i + 1) * CHW],
                    in0=ps[:, i * CHW:(i + 1) * CHW],
                    scalar=n1i[:, mt:mt + 1],
                    in1=srep2[:, cb * CB + i * CHW:cb * CB + (i + 1) * CHW],
                    op0=ALU.mult, op1=ALU.mult)
                if i == 1:
                    nc.sync.dma_start(
                        out=outb_ap[cb, mt * 128:(mt + 1) * 128, :],
                        in_=ot[:])

            x2cb = x2cb0
            x2nx = None
            sq_next = None
            pairs_next = None
            acc_next = None
            deferred = []
            for cb in range(NCB):
                csl_base = cb * CB
                if cb < NCB - 1:
                    # chain: a 1-elem gpsimd copy of THIS block's last
                    # k-tile; the next block's dma_starts sit behind it
                    # on the gpsimd queue, so input blocks never compete
                    # with each other for HBM bandwidth
                    g = vec.tile([128, 1], bf16, tag="gate",
                                 name=f"gate{cb}")
                    nc.gpsimd.tensor_copy(g[:], x2cb[:, KT - 1, CB - 1:CB])
                    x2nx = x2p.tile([128, KT, CB], bf16, tag="x2cb",
                                    name=f"x2cb{cb + 1}")
                    for k in range(KT):
                        nc.gpsimd.dma_start(out=x2nx[:, k, :],
                                            in_=x2b_ap[cb + 1, k])
                for mt in range(NMT):
                    ps = cps.tile([128, CB], f32, tag="c_ps", name="c_ps")
                    if not (cb == 0 and mt <= mt_flush):
                        ot = ost.tile([128, CB], f32, tag="c_ot",
                                      name="c_ot")
                    for i in range(2):
                        for k in range(KT):
                            nc.tensor.matmul(
                                ps[:, i * CHW:(i + 1) * CHW],
                                x1n[:, k, mt * 128:(mt + 1) * 128],
                                x2cb[:, k, i * CHW:(i + 1) * CHW],
                                start=(k == 0), stop=(k == KT - 1))
                        if cb == 0 and mt <= mt_flush:
                            deferred.append((ps, i, mt))
                        else:
                            _drain(ps, ot, i, mt, cb)
                    if cb == 0 and mt == mt_flush:
                        # cb0's own column norms, then the deferred drains
                        b_norm_reduce(acc_cb0, 0, "h")
                        dot = None
                        for (dps, di, dmt) in deferred:
                            if di == 0:
                                dot = ost.tile([128, CB], f32, tag="c_ot",
                                               name=f"c_ot_d{dmt}")
                            _drain(dps, dot, di, dmt, cb)
                        deferred = []
                    if cb < NCB - 1:
                        if mt == mt_sq:
                            # squares + pair-adds for cb+1
                            sq_next = sqp.tile([128, KT, CB], bf16,
                                               tag="sq2",
                                               name=f"sq2b{cb + 1}")
                            nc.scalar.activation(sq_next[:, 0:KH, :],
                                                 x2nx[:, 0:KH, :], AF.Square)
                            if KH < KT:
                                nc.scalar.activation(sq_next[:, KH:KT, :],
                                                     x2nx[:, KH:KT, :],
                                                     AF.Square)
                            pairs_next = []
                            pc = 0
                            for k2 in range(0, KT - 1, 2):
                                tt = trp.tile([128, CB], bf16,
                                              tag=f"tr{pc}",
                                              name=f"tr{pc}b{cb + 1}")
                                nc.vector.tensor_add(tt[:],
                                                     sq_next[:, k2, :],
                                                     sq_next[:, k2 + 1, :])
                                pairs_next.append(tt[:])
                                pc += 1
                            if KT % 2 == 1:
                                pairs_next.append(sq_next[:, KT - 1, :])
                        if mt == mt_fin:
                            # final tree levels for cb+1
                            acc_next = tree_reduce(pairs_next,
                                                   f"b{cb + 1}",
                                                   (KT + 1) // 2)
                        if mt == mt_red:
                            # cb+1's ones-matmuls + sqrt/recip -> srep2
                            b_norm_reduce(acc_next, cb + 1, f"b{cb + 1}")
                if cb < NCB - 1:
                    x2cb = x2nx

    nc.compile()
    return nc


def _get_nc(n_slab, m, d):
    key = (n_slab, m, d)
    if key not in _CACHE:
        _CACHE[key] = _build(n_slab, m, d)
    return _CACHE[key]


def _in_maps(x1, x2, n_slab):
    d = x1.shape[1]
    m = x2.shape[0]
    KT = d // 128
    CB = 1024
    NCB = m // CB
    x1t = np.ascontiguousarray(x1.T)  # [d, n]
    # blocked x2: [NCB, KT, 128, CB] so each (cb, k) k-tile block is a
    # single contiguous 512KB read on device (host layout shuffle only)
    x2bl = np.ascontiguousarray(
        x2.T.reshape(KT, 128, NCB, CB).transpose(2, 0, 1, 3))
    return [
        {"x1t": np.ascontiguousarray(x1t[:, i * n_slab:(i + 1) * n_slab]),
         "x2b": x2bl}
        for i in range(N_CORES)
    ]


def kernel(x1, x2):
    from concourse.bass_utils import run_bass_kernel_spmd

    x1 = np.asarray(x1, dtype=np.float32)
    x2 = np.asarray(x2, dtype=np.float32)
    n, d = x1.shape
    m, d2 = x2.shape
    assert d == d2 and n % N_CORES == 0
    n_slab = n // N_CORES

    nc = _get_nc(n_slab, m, d)
    res = run_bass_kernel_spmd(nc, _in_maps(x1, x2, n_slab),
                               core_ids=list(range(N_CORES)))
    # outb is [NCB, n_slab, CB] blocked; un-block to [n_slab, m]
    outs = [res.results[i]["outb"].transpose(1, 0, 2).reshape(n_slab, m)
            for i in range(N_CORES)]
    return np.ascontiguousarray(np.concatenate(outs, axis=0))


if __name__ == "__main__":
    # small-shape self test
    rng = np.random.default_rng(0)
    n, m, d = 1024, 2048, 256
    x1 = rng.standard_normal((n, d), dtype=np.float32)
    x2 = rng.standard_normal((m, d), dtype=np.float32)
    got = kernel(x1, x2)
    x1n = x1 / np.linalg.norm(x1, axis=1, keepdims=True)
    x2n = x2 / np.linalg.norm(x2, axis=1, keepdims=True)
    want = (x1n @ x2n.T) / TEMP
    rel = np.linalg.norm(got - want) / np.linalg.norm(want)
    print("rel l2 err:", rel)
    print("max abs err:", np.abs(got - want).max(), "scale:", np.abs(want).max())
